# revision 24
# baseline (speedup 1.0000x reference)
"""Bass/Trainium2 kernel for BilinearlyModulatedAttention.

Sharding: 8 cores = 2 (batch) x 4 (head groups of 4 heads).
Each core computes, for its batch b and heads [4g, 4g+4):
  QT/KT (feature-major, d x T), V (token-major), bilinear gate, causal
  softmax in transposed layout (scores[s, t]), PV with a ones-column
  giving softmax denominators, normalization, and a partial output
  projection Y_partial = O^T.T @ W_out[rows]. Host sums the 4 partials
  per batch and adds b_out.

Key layout/HW notes:
 - scores are computed TRANSPOSED (s on partitions, t on free dim) so the
   softmax denominator sum_s e[s,t] falls out of the PV matmul via an
   appended ones-column in the stationary operand (M=65).
 - no max-subtraction in softmax: scores are ~N(0,0.4), exp is safe.
 - all matmuls use float32r (1 PE cycle/row vs 4 for float32, ~1.6e-4
   matmul accuracy); use_f32r=False falls back to exact float32.
 - a PSUM bank must only ever receive matmuls of ONE contraction
   row-group base (mixing base-0 / base-64 writes corrupts results), so
   base-64 matmuls (odd heads' scores + gates) get a dedicated pool.
 - custom-DVE ops and partition_broadcast require partition-base-0 APs.
 - sigmoid is computed as 0.5*tanh(x/2)+0.5 (tanh shares the ACT table
   set with exp, avoiding ~2.7us table switches).
"""

import sys

if "/opt/trn_rl_repo" not in sys.path:
    sys.path.insert(0, "/opt/trn_rl_repo")

import numpy as np

D_MODEL = 1024
N_HEADS = 16
D_HEAD = 64
B = 2
T_FULL = 2048
N_CORES = 8
H_LOC = N_HEADS // (N_CORES // B)  # 4 heads per core

_LDW_PATCHED = False


def _patch_ldw_opt():
    """Compile walrus with --enable-ldw-opt=true (elides redundant
    LDWEIGHTS reloads). Wraps concourse.bass_utils.run_command."""
    global _LDW_PATCHED
    if _LDW_PATCHED:
        return
    import concourse.bass_utils as BU
    orig = BU.run_command

    def run_patched(argv, **kw):
        argv = [a.replace("--enable-ldw-opt=false", "--enable-ldw-opt=true")
                if isinstance(a, str) else a for a in argv]
        return orig(argv, **kw)

    BU.run_command = run_patched
    _LDW_PATCHED = True


def build_nc(T=T_FULL, D=D_MODEL, h_loc=H_LOC, dh=D_HEAD, W=512,
             use_f32r=True):
    """Build the Bass module for one core's shard. Returns (nc, meta)."""
    import concourse.bass as bass
    import concourse.mybir as mybir
    import concourse.tile as tile
    from concourse import bacc
    from contextlib import ExitStack
    from collections import deque

    f32 = mybir.dt.float32
    fm = mybir.dt.float32r if use_f32r else f32
    AF = mybir.ActivationFunctionType
    ALU = mybir.AluOpType

    KN = D // 128            # k-tiles for the qkv projections
    TT = T // 128            # 128-token tiles
    assert T % W == 0 and W == 512
    NCH = T // W             # chunks
    W128 = W // 128          # s-tiles per chunk (4)
    DHL = h_loc * dh         # local head dim total (256)
    NP = h_loc // 2          # head pairs
    KO = DHL // 128          # out-proj k-tiles (2)
    VGW = dh + 1             # vg last dim: 64 V cols + ones col
    SCALE = 1.0 / float(np.sqrt(dh))

    nc = bacc.Bacc("TRN2", target_bir_lowering=False, debug=False)

    xt_d = nc.dram_tensor("xt", (128, KN, T), fm, kind="ExternalInput")
    wq_d = nc.dram_tensor("wq", (128, KN, DHL), fm, kind="ExternalInput")
    wk_d = nc.dram_tensor("wk", (128, KN, DHL), fm, kind="ExternalInput")
    wv_d = nc.dram_tensor("wv", (128, KN, DHL), fm, kind="ExternalInput")
    wg_d = nc.dram_tensor("wg", (128, DHL), fm, kind="ExternalInput")
    wo_d = nc.dram_tensor("wo", (128, KO, D), fm, kind="ExternalInput")
    mask_d = nc.dram_tensor("mask", (128, 128), fm, kind="ExternalInput")
    ones_d = nc.dram_tensor("ones", (128, TT), fm, kind="ExternalInput")
    y_d = nc.dram_tensor("y", (T, D), f32, kind="ExternalOutput")

    with ExitStack() as ctx:
        tc = ctx.enter_context(tile.TileContext(nc))
        sb_w = ctx.enter_context(tc.tile_pool(name="wts", bufs=1))
        sb_big = ctx.enter_context(tc.tile_pool(name="big", bufs=1))
        sb_e = ctx.enter_context(tc.tile_pool(name="e", bufs=3))
        sb_sig = ctx.enter_context(tc.tile_pool(name="sig", bufs=2))
        sb_nrm = ctx.enter_context(tc.tile_pool(name="nrm", bufs=1))
        sb_y = ctx.enter_context(tc.tile_pool(name="ysb", bufs=2))
        ps_b0 = ctx.enter_context(
            tc.tile_pool(name="psb0", bufs=2, space=bass.MemorySpace.PSUM))
        ps_b64 = ctx.enter_context(
            tc.tile_pool(name="psb64", bufs=2, space=bass.MemorySpace.PSUM))
        ps_u = ctx.enter_context(
            tc.tile_pool(name="psu", bufs=2, space=bass.MemorySpace.PSUM))

        # ---- persistent SBUF tensors ----
        xt = sb_big.tile([128, KN, T], fm, tag="xt")
        wq = sb_w.tile([128, KN, DHL], fm, tag="wq")
        wk = sb_w.tile([128, KN, DHL], fm, tag="wk")
        wv = sb_w.tile([128, KN, DHL], fm, tag="wv")
        wg = sb_w.tile([128, DHL], fm, tag="wg")
        wo = sb_w.tile([128, KO, D], fm, tag="wo")
        msk = sb_w.tile([128, 128], fm, tag="msk")
        qt = [sb_big.tile([128, T], fm, tag=f"qt{p}", name=f"qt{p}")
              for p in range(NP)]
        kt = [sb_big.tile([128, T], fm, tag=f"kt{p}", name=f"kt{p}")
              for p in range(NP)]
        ot = [sb_big.tile([128, T], fm, tag=f"ot{p}", name=f"ot{p}")
              for p in range(NP)]
        vg = sb_big.tile([128, TT, h_loc, VGW], fm, tag="vg")

        # ---- input DMAs: interleave per-k weight planes with xt k-planes
        # so the first matmuls can start within a few microseconds ----
        for k in range(KN):
            nc.sync.dma_start(wq[:, k, :], wq_d[:, k, :])
            nc.sync.dma_start(wk[:, k, :], wk_d[:, k, :])
            nc.sync.dma_start(xt[:, k, :], xt_d[:, k, :])
        nc.sync.dma_start(wv[:], wv_d[:])
        nc.sync.dma_start(wg[:], wg_d[:])
        nc.sync.dma_start(msk[:], mask_d[:])
        for s in range(h_loc):
            nc.sync.dma_start(vg[:, :, s, dh], ones_d[:])
        nc.sync.dma_start(wo[:], wo_d[:])

        # ---- phase-A jobs ----
        def qk_job(w_sb, dst, p, c, nch=1):
            # nch chunks share each k's LDWEIGHTS (consecutive same-lhsT
            # matmuls are elided by --enable-ldw-opt=true)
            pss = [ps_b0.tile([128, W], f32, tag="b0", name=f"qkps{cc}")
                   for cc in range(nch)]
            for k in range(KN):
                for cc in range(nch):
                    nc.tensor.matmul(
                        pss[cc][:], w_sb[:, k, 128 * p:128 * p + 128],
                        xt[:, k, (c + cc) * W:(c + cc + 1) * W],
                        start=(k == 0), stop=(k == KN - 1),
                        skip_group_check=True)
            for cc in range(nch):
                nc.vector.tensor_copy(
                    dst[:, (c + cc) * W:(c + cc + 1) * W], pss[cc][:])

        def vg_job(ti):
            # one base-0 psum tile: V in cols [0:DHL), j=0 gates in
            # [DHL:DHL+128). j=1 gates go to the base-64 pool.
            vps = ps_b0.tile([128, W], f32, tag="b0")
            for k in range(KN):
                nc.tensor.matmul(
                    vps[:, :DHL],
                    xt[:, k, 128 * ti:128 * ti + 128],
                    wv[:, k, :],
                    start=(k == 0), stop=(k == KN - 1),
                    skip_group_check=True)
            for p in range(NP):
                h = 2 * p
                nc.tensor.matmul(
                    vps[:, DHL + 64 * p:DHL + 64 * p + 64],
                    qt[p][0:64, 128 * ti:128 * ti + 128],
                    wg[0:64, dh * h:dh * h + dh],
                    start=True, stop=True, skip_group_check=True)
            gps1 = ps_b64.tile([128, W], f32, tag="b64")
            for p in range(NP):
                h = 2 * p + 1
                nc.tensor.matmul(
                    gps1[:, 64 * p:64 * p + 64],
                    qt[p][64:128, 128 * ti:128 * ti + 128],
                    wg[64:128, dh * h:dh * h + dh],
                    start=True, stop=True, skip_group_check=True)
            # sigmoid(x) = 0.5*tanh(x/2) + 0.5 (stays in the exp table set)
            sig = sb_sig.tile([128, DHL], f32, tag="sig")
            sig4 = sig[:].rearrange("p (a b c) -> p a b c", a=NP, b=2)
            nc.scalar.activation(
                sig4[:, :, 0, :],
                vps[:, DHL:DHL + 128].rearrange("p (a c) -> p a c", a=NP),
                AF.Tanh, scale=0.5)
            nc.scalar.activation(
                sig4[:, :, 1, :],
                gps1[:, 0:128].rearrange("p (a c) -> p a c", a=NP),
                AF.Tanh, scale=0.5)
            nc.vector.tensor_scalar(sig[:], sig[:], 0.5, 0.5,
                                    ALU.mult, ALU.add)
            nc.vector.tensor_mul(
                vg[:, ti, :, 0:dh],
                vps[:, :DHL].rearrange("p (h d) -> p h d", h=h_loc),
                sig[:].rearrange("p (h d) -> p h d", h=h_loc))

        # ---- phase-B inner iteration ----
        def b_iter(c, p, i, UA, UB, S):
            base = c * W128
            off = 128 * (i - base) if i >= base else 0
            sA = ps_b0.tile([128, W], f32, tag="b0")
            sB = ps_b64.tile([128, W], f32, tag="b64")
            for j, sps in ((0, sA), (1, sB)):
                nc.tensor.matmul(
                    sps[:, off:W],
                    kt[p][64 * j:64 * j + 64, 128 * i:128 * i + 128],
                    qt[p][64 * j:64 * j + 64, c * W + off:(c + 1) * W],
                    start=True, stop=True)
            es = []
            for sps in (sA, sB):
                e = sb_e.tile([128, W], fm, tag="e")
                nc.scalar.activation(e[:, off:W], sps[:, off:W], AF.Exp,
                                     scale=SCALE)
                if i >= base:
                    nc.vector.tensor_mul(e[:, off:off + 128],
                                         e[:, off:off + 128], msk[:])
                es.append(e)
            last_i = min(S - 1, base + W128 - 1)
            for j, (e, U) in ((0, (es[0], UA)), (1, (es[1], UB))):
                nc.tensor.matmul(
                    U[0:65, off:W],
                    vg[:, i, 2 * p + j, 0:65],
                    e[:, off:W],
                    start=(i == 0), stop=(i == last_i),
                    skip_group_check=True)

        def normalize(c, p, UA, UB):
            # Denominator rows live at partition 64; custom-DVE ops and
            # partition_broadcast need base-0 APs, so bounce them through
            # a cross-partition SBUF DMA.
            dtA = sb_nrm.tile([65, W], f32, tag="dtA")
            dtB = sb_nrm.tile([65, W], f32, tag="dtB")
            nc.vector.tensor_copy(dtA[64:65, :], UA[64:65, :])
            nc.vector.tensor_copy(dtB[64:65, :], UB[64:65, :])
            den = sb_nrm.tile([2, W], f32, tag="den")
            nc.sync.dma_start(den[0:1, :], dtA[64:65, :])
            nc.sync.dma_start(den[1:2, :], dtB[64:65, :])
            rec = sb_nrm.tile([2, W], f32, tag="rec")
            nc.vector.reciprocal_approx_fast(rec[:], den[:])
            recB = sb_nrm.tile([1, W], f32, tag="recB")
            nc.sync.dma_start(recB[:], rec[1:2, :])
            bcA = sb_nrm.tile([64, W], f32, tag="bcA")
            bcB = sb_nrm.tile([64, W], f32, tag="bcB")
            nc.gpsimd.partition_broadcast(bcA[:], rec[0:1, :])
            nc.gpsimd.partition_broadcast(bcB[:], recB[:])
            nc.vector.tensor_mul(ot[p][0:64, c * W:(c + 1) * W],
                                 UA[0:64, :], bcA[:])
            obB = sb_nrm.tile([64, W], fm, tag="obB")
            nc.vector.tensor_mul(obB[:], UB[0:64, :], bcB[:])
            nc.sync.dma_start(ot[p][64:128, c * W:(c + 1) * W], obB[:])

        # ---- phase-C job (one 128-token tile x one 512-col slab) ----
        def c_job(tt):
            NSL = D // 512
            yps = [ps_b0.tile([128, W], f32, tag="b0", name=f"yp{n}")
                   for n in range(NSL)]
            for kt_i in range(KO):
                for n in range(NSL):
                    nc.tensor.matmul(
                        yps[n][:],
                        ot[kt_i][:, 128 * tt:128 * tt + 128],
                        wo[:, kt_i, n * 512:(n + 1) * 512],
                        start=(kt_i == 0), stop=(kt_i == KO - 1),
                        skip_group_check=True)
            for n in range(NSL):
                ysb = sb_y.tile([128, W], f32, tag="ysb", name=f"ysb{n}")
                nc.vector.tensor_copy(ysb[:], yps[n][:])
                nc.sync.dma_start(
                    y_d[128 * tt:128 * tt + 128, n * 512:(n + 1) * 512],
                    ysb[:])

        def b_scores(c, p, i, S):
            base = c * W128
            off = 128 * (i - base) if i >= base else 0
            sA = ps_b0.tile([128, W], f32, tag="b0")
            sB = ps_b64.tile([128, W], f32, tag="b64")
            for j, sps in ((0, sA), (1, sB)):
                nc.tensor.matmul(
                    sps[:, off:W],
                    kt[p][64 * j:64 * j + 64, 128 * i:128 * i + 128],
                    qt[p][64 * j:64 * j + 64, c * W + off:(c + 1) * W],
                    start=True, stop=True)
            es = []
            for sps in (sA, sB):
                e = sb_e.tile([128, W], fm, tag="e")
                nc.scalar.activation(e[:, off:W], sps[:, off:W], AF.Exp,
                                     scale=SCALE)
                if i >= base:
                    nc.vector.tensor_mul(e[:, off:off + 128],
                                         e[:, off:off + 128], msk[:])
                es.append(e)
            return es, off

        def b_pv(c, p, i, UA, UB, S, es, off):
            base = c * W128
            last_i = min(S - 1, base + W128 - 1)
            for j, (e, U) in ((0, (es[0], UA)), (1, (es[1], UB))):
                nc.tensor.matmul(
                    U[0:65, off:W],
                    vg[:, i, 2 * p + j, 0:65],
                    e[:, off:W],
                    start=(i == 0), stop=(i == last_i),
                    skip_group_check=True)

        # ---- emission schedule ----
        # urgent fillers (projections/vg the next chunks depend on) are
        # popped every B iteration; lazy fillers (out-proj jobs) are
        # rationed to every other iteration so the late, filler-starved
        # chunks still get PE work (keeps HAM warm).
        urgent = deque()
        lazy = deque()

        for p in range(NP):
            qk_job(wq, qt[p], p, 0, nch=min(2, NCH))
        for p in range(NP):
            qk_job(wk, kt[p], p, 0, nch=min(2, NCH))
        for ti in range(W128):
            vg_job(ti)

        if NCH > 2:
            for p in range(NP):
                urgent.append(
                    lambda p=p: qk_job(wq, qt[p], p, 2, NCH - 2))
            for p in range(NP):
                urgent.append(
                    lambda p=p: qk_job(wk, kt[p], p, 2, NCH - 2))
        for ti in range(W128, TT):
            urgent.append(lambda ti=ti: vg_job(ti))

        it = 0
        for c in range(NCH):
            S = (c + 1) * W128
            for p in range(NP):
                UA = ps_u.tile([65, W], f32, tag="UA", name="UA")
                UB = ps_u.tile([65, W], f32, tag="UB", name="UB")
                for i in range(S):
                    es, off = b_scores(c, p, i, S)
                    b_pv(c, p, i, UA, UB, S, es, off)
                    if urgent:
                        urgent.popleft()()
                    elif lazy and it % 2 == 0:
                        lazy.popleft()()
                    it += 1
                normalize(c, p, UA, UB)
            for tt in range(c * W128, (c + 1) * W128):
                lazy.append(lambda tt=tt: c_job(tt))
        while urgent:
            urgent.popleft()()
        while lazy:
            lazy.popleft()()

    nc.compile()
    meta = dict(T=T, D=D, h_loc=h_loc, dh=dh, W=W)
    return nc, meta


def prepare_core_inputs(x, W_qkv, b_qkv, W_g, W_out, b_out,
                        T=T_FULL, D=D_MODEL, h_loc=H_LOC, dh=D_HEAD):
    """Host-side sharding: returns list of per-core input dicts."""
    x = np.asarray(x, dtype=np.float32)
    W_qkv = np.asarray(W_qkv, dtype=np.float32)
    W_g = np.asarray(W_g, dtype=np.float32)
    W_out = np.asarray(W_out, dtype=np.float32)
    KN = D // 128
    DHL = h_loc * dh
    KO = DHL // 128
    n_groups = N_CORES // B
    mask = np.ascontiguousarray(
        (np.arange(128)[:, None] <= np.arange(128)[None, :]).astype(np.float32))

    in_maps = []
    for core in range(N_CORES):
        b, g = divmod(core, n_groups)
        cols = slice(DHL * g, DHL * (g + 1))
        xt = np.ascontiguousarray(
            x[b].T.reshape(KN, 128, T).transpose(1, 0, 2))
        wq = np.ascontiguousarray(
            W_qkv[:, 0 * D:1 * D][:, cols].reshape(KN, 128, DHL).transpose(1, 0, 2))
        wk = np.ascontiguousarray(
            W_qkv[:, 1 * D:2 * D][:, cols].reshape(KN, 128, DHL).transpose(1, 0, 2))
        wv = np.ascontiguousarray(
            W_qkv[:, 2 * D:3 * D][:, cols].reshape(KN, 128, DHL).transpose(1, 0, 2))
        wgh = np.zeros((128, DHL), dtype=np.float32)
        for lh in range(h_loc):
            j = lh % 2
            wgh[64 * j:64 * j + 64, dh * lh:dh * lh + dh] = W_g[h_loc * g + lh]
        wo = np.ascontiguousarray(
            W_out[DHL * g:DHL * (g + 1), :].reshape(KO, 128, D).transpose(1, 0, 2))
        in_maps.append({
            "xt": xt, "wq": wq, "wk": wk, "wv": wv,
            "wg": wgh, "wo": wo, "mask": mask,
            "ones": np.ones((128, T // 128), dtype=np.float32),
        })
    return in_maps


def gather_output(results, b_out):
    """Sum the per-core partial projections into the full output."""
    n_groups = N_CORES // B
    b_out = np.asarray(b_out, dtype=np.float32)
    outs = []
    for b in range(B):
        acc = None
        for g in range(n_groups):
            part = results[b * n_groups + g]["y"]
            acc = part.copy() if acc is None else acc + part
        outs.append(acc + b_out[None, :])
    return np.stack(outs, axis=0)


_BUILD_CACHE = {}


def _get_nc():
    key = (T_FULL, D_MODEL, H_LOC, D_HEAD)
    if key not in _BUILD_CACHE:
        _BUILD_CACHE[key] = build_nc()
    return _BUILD_CACHE[key]


def kernel(x, W_qkv, b_qkv, W_g, W_out, b_out):
    _patch_ldw_opt()
    from concourse.bass_utils import run_bass_kernel_spmd

    b_qkv = np.asarray(b_qkv, dtype=np.float32)
    assert not np.any(b_qkv), "nonzero b_qkv not supported by this build"
    nc, _ = _get_nc()
    in_maps = prepare_core_inputs(x, W_qkv, b_qkv, W_g, W_out, b_out)
    res = run_bass_kernel_spmd(nc, in_maps, core_ids=list(range(N_CORES)))
    return gather_output(res.results, b_out).astype(np.float32)


# revision 25
# speedup vs baseline: 1.0629x; 1.0629x over previous
"""Bass/Trainium2 kernel for BilinearlyModulatedAttention.

Sharding: 8 cores = 2 (batch) x 4 (head groups of 4 heads).
Each core computes, for its batch b and heads [4g, 4g+4):
  QT/KT (feature-major, d x T), V (token-major), bilinear gate, causal
  softmax in transposed layout (scores[s, t]), PV with a ones-column
  giving softmax denominators, normalization, and a partial output
  projection Y_partial = O^T.T @ W_out[rows]. Host sums the 4 partials
  per batch and adds b_out.

Key layout/HW notes:
 - scores are computed TRANSPOSED (s on partitions, t on free dim) so the
   softmax denominator sum_s e[s,t] falls out of the PV matmul via an
   appended ones-column in the stationary operand (M=65).
 - no max-subtraction in softmax: scores are ~N(0,0.4), exp is safe.
 - all matmuls use float32r (1 PE cycle/row vs 4 for float32, ~1.6e-4
   matmul accuracy); use_f32r=False falls back to exact float32.
 - a PSUM bank must only ever receive matmuls of ONE contraction
   row-group base (mixing base-0 / base-64 writes corrupts results), so
   base-64 matmuls (odd heads' scores + gates) get a dedicated pool.
 - custom-DVE ops and partition_broadcast require partition-base-0 APs.
 - sigmoid is computed as 0.5*tanh(x/2)+0.5 (tanh shares the ACT table
   set with exp, avoiding ~2.7us table switches).
"""

import sys

if "/opt/trn_rl_repo" not in sys.path:
    sys.path.insert(0, "/opt/trn_rl_repo")

import numpy as np

D_MODEL = 1024
N_HEADS = 16
D_HEAD = 64
B = 2
T_FULL = 2048
N_CORES = 8
H_LOC = N_HEADS // (N_CORES // B)  # 4 heads per core

_LDW_PATCHED = False


def _patch_ldw_opt():
    """Compile walrus with --enable-ldw-opt=true (elides redundant
    LDWEIGHTS reloads). Wraps concourse.bass_utils.run_command."""
    global _LDW_PATCHED
    if _LDW_PATCHED:
        return
    import concourse.bass_utils as BU
    orig = BU.run_command

    def run_patched(argv, **kw):
        argv = [a.replace("--enable-ldw-opt=false", "--enable-ldw-opt=true")
                if isinstance(a, str) else a for a in argv]
        return orig(argv, **kw)

    BU.run_command = run_patched
    _LDW_PATCHED = True


def build_nc(T=T_FULL, D=D_MODEL, h_loc=H_LOC, dh=D_HEAD, W=512,
             use_f32r=True):
    """Build the Bass module for one core's shard. Returns (nc, meta)."""
    import concourse.bass as bass
    import concourse.mybir as mybir
    import concourse.tile as tile
    from concourse import bacc
    from contextlib import ExitStack
    from collections import deque

    f32 = mybir.dt.float32
    fm = mybir.dt.float32r if use_f32r else f32
    AF = mybir.ActivationFunctionType
    ALU = mybir.AluOpType

    KN = D // 128            # k-tiles for the qkv projections
    TT = T // 128            # 128-token tiles
    assert T % W == 0 and W == 512
    NCH = T // W             # chunks
    W128 = W // 128          # s-tiles per chunk (4)
    DHL = h_loc * dh         # local head dim total (256)
    NP = h_loc // 2          # head pairs
    KO = DHL // 128          # out-proj k-tiles (2)
    VGW = dh + 1             # vg last dim: 64 V cols + ones col
    SCALE = 1.0 / float(np.sqrt(dh))

    nc = bacc.Bacc("TRN2", target_bir_lowering=False, debug=False)

    xt_d = nc.dram_tensor("xt", (128, KN, T), fm, kind="ExternalInput")
    wq_d = nc.dram_tensor("wq", (128, KN, DHL), fm, kind="ExternalInput")
    wk_d = nc.dram_tensor("wk", (128, KN, DHL), fm, kind="ExternalInput")
    wv_d = nc.dram_tensor("wv", (128, KN, DHL), fm, kind="ExternalInput")
    wg_d = nc.dram_tensor("wg", (128, DHL), fm, kind="ExternalInput")
    wo_d = nc.dram_tensor("wo", (128, KO, D), fm, kind="ExternalInput")
    mask_d = nc.dram_tensor("mask", (128, 128), fm, kind="ExternalInput")
    ones_d = nc.dram_tensor("ones", (128, TT), fm, kind="ExternalInput")
    y_d = nc.dram_tensor("y", (T, D), f32, kind="ExternalOutput")

    with ExitStack() as ctx:
        tc = ctx.enter_context(tile.TileContext(nc))
        sb_w = ctx.enter_context(tc.tile_pool(name="wts", bufs=1))
        sb_big = ctx.enter_context(tc.tile_pool(name="big", bufs=1))
        sb_e = ctx.enter_context(tc.tile_pool(name="e", bufs=4))
        sb_sig = ctx.enter_context(tc.tile_pool(name="sig", bufs=2))
        sb_nrm = ctx.enter_context(tc.tile_pool(name="nrm", bufs=1))
        sb_y = ctx.enter_context(tc.tile_pool(name="ysb", bufs=2))
        ps_b0 = ctx.enter_context(
            tc.tile_pool(name="psb0", bufs=2, space=bass.MemorySpace.PSUM))
        ps_b64 = ctx.enter_context(
            tc.tile_pool(name="psb64", bufs=2, space=bass.MemorySpace.PSUM))
        ps_u = ctx.enter_context(
            tc.tile_pool(name="psu", bufs=2, space=bass.MemorySpace.PSUM))

        # ---- persistent SBUF tensors ----
        xt = sb_big.tile([128, KN, T], fm, tag="xt")
        wq = sb_w.tile([128, KN, DHL], fm, tag="wq")
        wk = sb_w.tile([128, KN, DHL], fm, tag="wk")
        wv = sb_w.tile([128, KN, DHL], fm, tag="wv")
        wg = sb_w.tile([128, DHL], fm, tag="wg")
        wo = sb_w.tile([128, KO, D], fm, tag="wo")
        msk = sb_w.tile([128, 128], fm, tag="msk")
        qt = [sb_big.tile([128, T], fm, tag=f"qt{p}", name=f"qt{p}")
              for p in range(NP)]
        kt = [sb_big.tile([128, T], fm, tag=f"kt{p}", name=f"kt{p}")
              for p in range(NP)]
        ot = [sb_big.tile([128, T], fm, tag=f"ot{p}", name=f"ot{p}")
              for p in range(NP)]
        vg = sb_big.tile([128, TT, h_loc, VGW], fm, tag="vg")

        # ---- input DMAs: interleave per-k weight planes with xt k-planes
        # so the first matmuls can start within a few microseconds ----
        for k in range(KN):
            nc.sync.dma_start(wq[:, k, :], wq_d[:, k, :])
            nc.sync.dma_start(wk[:, k, :], wk_d[:, k, :])
            nc.sync.dma_start(xt[:, k, :], xt_d[:, k, :])
        nc.sync.dma_start(wv[:], wv_d[:])
        nc.sync.dma_start(wg[:], wg_d[:])
        nc.sync.dma_start(msk[:], mask_d[:])
        for s in range(h_loc):
            nc.sync.dma_start(vg[:, :, s, dh], ones_d[:])
        nc.sync.dma_start(wo[:], wo_d[:])

        # ---- phase-A jobs ----
        def qk_job(w_sb, dst, p, c, nch=1):
            # nch chunks share each k's LDWEIGHTS (consecutive same-lhsT
            # matmuls are elided by --enable-ldw-opt=true)
            pss = [ps_b0.tile([128, W], f32, tag="b0", name=f"qkps{cc}")
                   for cc in range(nch)]
            for k in range(KN):
                for cc in range(nch):
                    nc.tensor.matmul(
                        pss[cc][:], w_sb[:, k, 128 * p:128 * p + 128],
                        xt[:, k, (c + cc) * W:(c + cc + 1) * W],
                        start=(k == 0), stop=(k == KN - 1),
                        skip_group_check=True)
            for cc in range(nch):
                nc.vector.tensor_copy(
                    dst[:, (c + cc) * W:(c + cc + 1) * W], pss[cc][:])

        def vg_job(ti):
            # one base-0 psum tile: V in cols [0:DHL), j=0 gates in
            # [DHL:DHL+128). j=1 gates go to the base-64 pool.
            vps = ps_b0.tile([128, W], f32, tag="b0")
            for k in range(KN):
                nc.tensor.matmul(
                    vps[:, :DHL],
                    xt[:, k, 128 * ti:128 * ti + 128],
                    wv[:, k, :],
                    start=(k == 0), stop=(k == KN - 1),
                    skip_group_check=True)
            for p in range(NP):
                h = 2 * p
                nc.tensor.matmul(
                    vps[:, DHL + 64 * p:DHL + 64 * p + 64],
                    qt[p][0:64, 128 * ti:128 * ti + 128],
                    wg[0:64, dh * h:dh * h + dh],
                    start=True, stop=True, skip_group_check=True)
            gps1 = ps_b64.tile([128, W], f32, tag="b64")
            for p in range(NP):
                h = 2 * p + 1
                nc.tensor.matmul(
                    gps1[:, 64 * p:64 * p + 64],
                    qt[p][64:128, 128 * ti:128 * ti + 128],
                    wg[64:128, dh * h:dh * h + dh],
                    start=True, stop=True, skip_group_check=True)
            # sigmoid(x) = 0.5*tanh(x/2) + 0.5 (stays in the exp table set)
            sig = sb_sig.tile([128, DHL], f32, tag="sig")
            sig4 = sig[:].rearrange("p (a b c) -> p a b c", a=NP, b=2)
            nc.scalar.activation(
                sig4[:, :, 0, :],
                vps[:, DHL:DHL + 128].rearrange("p (a c) -> p a c", a=NP),
                AF.Tanh, scale=0.5)
            nc.scalar.activation(
                sig4[:, :, 1, :],
                gps1[:, 0:128].rearrange("p (a c) -> p a c", a=NP),
                AF.Tanh, scale=0.5)
            nc.vector.tensor_scalar(sig[:], sig[:], 0.5, 0.5,
                                    ALU.mult, ALU.add)
            nc.vector.tensor_mul(
                vg[:, ti, :, 0:dh],
                vps[:, :DHL].rearrange("p (h d) -> p h d", h=h_loc),
                sig[:].rearrange("p (h d) -> p h d", h=h_loc))

        # ---- phase-B inner iteration ----
        def b_iter(c, p, i, UA, UB, S):
            base = c * W128
            off = 128 * (i - base) if i >= base else 0
            sA = ps_b0.tile([128, W], f32, tag="b0")
            sB = ps_b64.tile([128, W], f32, tag="b64")
            for j, sps in ((0, sA), (1, sB)):
                nc.tensor.matmul(
                    sps[:, off:W],
                    kt[p][64 * j:64 * j + 64, 128 * i:128 * i + 128],
                    qt[p][64 * j:64 * j + 64, c * W + off:(c + 1) * W],
                    start=True, stop=True)
            es = []
            for sps in (sA, sB):
                e = sb_e.tile([128, W], fm, tag="e")
                nc.scalar.activation(e[:, off:W], sps[:, off:W], AF.Exp,
                                     scale=SCALE)
                if i >= base:
                    nc.vector.tensor_mul(e[:, off:off + 128],
                                         e[:, off:off + 128], msk[:])
                es.append(e)
            last_i = min(S - 1, base + W128 - 1)
            for j, (e, U) in ((0, (es[0], UA)), (1, (es[1], UB))):
                nc.tensor.matmul(
                    U[0:65, off:W],
                    vg[:, i, 2 * p + j, 0:65],
                    e[:, off:W],
                    start=(i == 0), stop=(i == last_i),
                    skip_group_check=True)

        def normalize(c, p, UA, UB):
            # Denominator rows live at partition 64; custom-DVE ops and
            # partition_broadcast need base-0 APs, so bounce them through
            # a cross-partition SBUF DMA.
            dtA = sb_nrm.tile([65, W], f32, tag="dtA")
            dtB = sb_nrm.tile([65, W], f32, tag="dtB")
            nc.vector.tensor_copy(dtA[64:65, :], UA[64:65, :])
            nc.vector.tensor_copy(dtB[64:65, :], UB[64:65, :])
            den = sb_nrm.tile([2, W], f32, tag="den")
            nc.sync.dma_start(den[0:1, :], dtA[64:65, :])
            nc.sync.dma_start(den[1:2, :], dtB[64:65, :])
            rec = sb_nrm.tile([2, W], f32, tag="rec")
            nc.vector.reciprocal_approx_fast(rec[:], den[:])
            recB = sb_nrm.tile([1, W], f32, tag="recB")
            nc.sync.dma_start(recB[:], rec[1:2, :])
            bcA = sb_nrm.tile([64, W], f32, tag="bcA")
            bcB = sb_nrm.tile([64, W], f32, tag="bcB")
            nc.gpsimd.partition_broadcast(bcA[:], rec[0:1, :])
            nc.gpsimd.partition_broadcast(bcB[:], recB[:])
            nc.vector.tensor_mul(ot[p][0:64, c * W:(c + 1) * W],
                                 UA[0:64, :], bcA[:])
            obB = sb_nrm.tile([64, W], fm, tag="obB")
            nc.vector.tensor_mul(obB[:], UB[0:64, :], bcB[:])
            nc.sync.dma_start(ot[p][64:128, c * W:(c + 1) * W], obB[:])

        # ---- phase-C job (one 128-token tile x one 512-col slab) ----
        def c_job(tt):
            NSL = D // 512
            yps = [ps_b0.tile([128, W], f32, tag="b0", name=f"yp{n}")
                   for n in range(NSL)]
            for kt_i in range(KO):
                for n in range(NSL):
                    nc.tensor.matmul(
                        yps[n][:],
                        ot[kt_i][:, 128 * tt:128 * tt + 128],
                        wo[:, kt_i, n * 512:(n + 1) * 512],
                        start=(kt_i == 0), stop=(kt_i == KO - 1),
                        skip_group_check=True)
            for n in range(NSL):
                ysb = sb_y.tile([128, W], f32, tag="ysb", name=f"ysb{n}")
                nc.vector.tensor_copy(ysb[:], yps[n][:])
                nc.sync.dma_start(
                    y_d[128 * tt:128 * tt + 128, n * 512:(n + 1) * 512],
                    ysb[:])

        def b_scores(c, p, i, S):
            base = c * W128
            off = 128 * (i - base) if i >= base else 0
            sA = ps_b0.tile([128, W], f32, tag="b0")
            sB = ps_b64.tile([128, W], f32, tag="b64")
            for j, sps in ((0, sA), (1, sB)):
                nc.tensor.matmul(
                    sps[:, off:W],
                    kt[p][64 * j:64 * j + 64, 128 * i:128 * i + 128],
                    qt[p][64 * j:64 * j + 64, c * W + off:(c + 1) * W],
                    start=True, stop=True)
            es = []
            for sps in (sA, sB):
                e = sb_e.tile([128, W], fm, tag="e")
                nc.scalar.activation(e[:, off:W], sps[:, off:W], AF.Exp,
                                     scale=SCALE)
                if i >= base:
                    nc.vector.tensor_mul(e[:, off:off + 128],
                                         e[:, off:off + 128], msk[:])
                es.append(e)
            return es, off

        def b_pv(c, p, i, UA, UB, S, es, off):
            base = c * W128
            last_i = min(S - 1, base + W128 - 1)
            for j, (e, U) in ((0, (es[0], UA)), (1, (es[1], UB))):
                nc.tensor.matmul(
                    U[0:65, off:W],
                    vg[:, i, 2 * p + j, 0:65],
                    e[:, off:W],
                    start=(i == 0), stop=(i == last_i),
                    skip_group_check=True)

        # ---- emission schedule ----
        # urgent fillers (projections/vg the next chunks depend on) are
        # popped every B iteration; lazy fillers (out-proj jobs) are
        # rationed to every other iteration so the late, filler-starved
        # chunks still get PE work (keeps HAM warm).
        urgent = deque()
        lazy = deque()

        for p in range(NP):
            qk_job(wq, qt[p], p, 0)
        for p in range(NP):
            qk_job(wk, kt[p], p, 0)
        for ti in range(W128):
            vg_job(ti)

        for c in range(1, NCH):
            for p in range(NP):
                urgent.append(lambda p=p, c=c: qk_job(wq, qt[p], p, c))
            for p in range(NP):
                urgent.append(lambda p=p, c=c: qk_job(wk, kt[p], p, c))
        for ti in range(W128, TT):
            urgent.append(lambda ti=ti: vg_job(ti))

        it = 0
        for c in range(NCH):
            S = (c + 1) * W128
            for p in range(NP):
                UA = ps_u.tile([65, W], f32, tag="UA", name="UA")
                UB = ps_u.tile([65, W], f32, tag="UB", name="UB")
                for i in range(S):
                    es, off = b_scores(c, p, i, S)
                    b_pv(c, p, i, UA, UB, S, es, off)
                    if urgent:
                        urgent.popleft()()
                    elif lazy and it % 2 == 0:
                        lazy.popleft()()
                    it += 1
                normalize(c, p, UA, UB)
            for tt in range(c * W128, (c + 1) * W128):
                lazy.append(lambda tt=tt: c_job(tt))
        while urgent:
            urgent.popleft()()
        while lazy:
            lazy.popleft()()

    nc.compile()
    meta = dict(T=T, D=D, h_loc=h_loc, dh=dh, W=W)
    return nc, meta


def prepare_core_inputs(x, W_qkv, b_qkv, W_g, W_out, b_out,
                        T=T_FULL, D=D_MODEL, h_loc=H_LOC, dh=D_HEAD):
    """Host-side sharding: returns list of per-core input dicts."""
    x = np.asarray(x, dtype=np.float32)
    W_qkv = np.asarray(W_qkv, dtype=np.float32)
    W_g = np.asarray(W_g, dtype=np.float32)
    W_out = np.asarray(W_out, dtype=np.float32)
    KN = D // 128
    DHL = h_loc * dh
    KO = DHL // 128
    n_groups = N_CORES // B
    mask = np.ascontiguousarray(
        (np.arange(128)[:, None] <= np.arange(128)[None, :]).astype(np.float32))

    in_maps = []
    for core in range(N_CORES):
        b, g = divmod(core, n_groups)
        cols = slice(DHL * g, DHL * (g + 1))
        xt = np.ascontiguousarray(
            x[b].T.reshape(KN, 128, T).transpose(1, 0, 2))
        wq = np.ascontiguousarray(
            W_qkv[:, 0 * D:1 * D][:, cols].reshape(KN, 128, DHL).transpose(1, 0, 2))
        wk = np.ascontiguousarray(
            W_qkv[:, 1 * D:2 * D][:, cols].reshape(KN, 128, DHL).transpose(1, 0, 2))
        wv = np.ascontiguousarray(
            W_qkv[:, 2 * D:3 * D][:, cols].reshape(KN, 128, DHL).transpose(1, 0, 2))
        wgh = np.zeros((128, DHL), dtype=np.float32)
        for lh in range(h_loc):
            j = lh % 2
            wgh[64 * j:64 * j + 64, dh * lh:dh * lh + dh] = W_g[h_loc * g + lh]
        wo = np.ascontiguousarray(
            W_out[DHL * g:DHL * (g + 1), :].reshape(KO, 128, D).transpose(1, 0, 2))
        in_maps.append({
            "xt": xt, "wq": wq, "wk": wk, "wv": wv,
            "wg": wgh, "wo": wo, "mask": mask,
            "ones": np.ones((128, T // 128), dtype=np.float32),
        })
    return in_maps


def gather_output(results, b_out):
    """Sum the per-core partial projections into the full output."""
    n_groups = N_CORES // B
    b_out = np.asarray(b_out, dtype=np.float32)
    outs = []
    for b in range(B):
        acc = None
        for g in range(n_groups):
            part = results[b * n_groups + g]["y"]
            acc = part.copy() if acc is None else acc + part
        outs.append(acc + b_out[None, :])
    return np.stack(outs, axis=0)


_BUILD_CACHE = {}


def _get_nc():
    key = (T_FULL, D_MODEL, H_LOC, D_HEAD)
    if key not in _BUILD_CACHE:
        _BUILD_CACHE[key] = build_nc()
    return _BUILD_CACHE[key]


def kernel(x, W_qkv, b_qkv, W_g, W_out, b_out):
    _patch_ldw_opt()
    from concourse.bass_utils import run_bass_kernel_spmd

    b_qkv = np.asarray(b_qkv, dtype=np.float32)
    assert not np.any(b_qkv), "nonzero b_qkv not supported by this build"
    nc, _ = _get_nc()
    in_maps = prepare_core_inputs(x, W_qkv, b_qkv, W_g, W_out, b_out)
    res = run_bass_kernel_spmd(nc, in_maps, core_ids=list(range(N_CORES)))
    return gather_output(res.results, b_out).astype(np.float32)


# revision 26
# speedup vs baseline: 1.0727x; 1.0092x over previous
"""Bass/Trainium2 kernel for BilinearlyModulatedAttention.

Sharding: 8 cores = 2 (batch) x 4 (head groups of 4 heads).
Each core computes, for its batch b and heads [4g, 4g+4):
  QT/KT (feature-major, d x T), V (token-major), bilinear gate, causal
  softmax in transposed layout (scores[s, t]), PV with a ones-column
  giving softmax denominators, normalization, and a partial output
  projection Y_partial = O^T.T @ W_out[rows]. Host sums the 4 partials
  per batch and adds b_out.

Key layout/HW notes:
 - scores are computed TRANSPOSED (s on partitions, t on free dim) so the
   softmax denominator sum_s e[s,t] falls out of the PV matmul via an
   appended ones-column in the stationary operand (M=65).
 - no max-subtraction in softmax: scores are ~N(0,0.4), exp is safe.
 - all matmuls use float32r (1 PE cycle/row vs 4 for float32, ~1.6e-4
   matmul accuracy); use_f32r=False falls back to exact float32.
 - a PSUM bank must only ever receive matmuls of ONE contraction
   row-group base (mixing base-0 / base-64 writes corrupts results), so
   base-64 matmuls (odd heads' scores + gates) get a dedicated pool.
 - custom-DVE ops and partition_broadcast require partition-base-0 APs.
 - sigmoid is computed as 0.5*tanh(x/2)+0.5 (tanh shares the ACT table
   set with exp, avoiding ~2.7us table switches).
"""

import sys

if "/opt/trn_rl_repo" not in sys.path:
    sys.path.insert(0, "/opt/trn_rl_repo")

import numpy as np

D_MODEL = 1024
N_HEADS = 16
D_HEAD = 64
B = 2
T_FULL = 2048
N_CORES = 8
H_LOC = N_HEADS // (N_CORES // B)  # 4 heads per core

_LDW_PATCHED = False


def _patch_ldw_opt():
    """Compile walrus with --enable-ldw-opt=true (elides redundant
    LDWEIGHTS reloads). Wraps concourse.bass_utils.run_command."""
    global _LDW_PATCHED
    if _LDW_PATCHED:
        return
    import concourse.bass_utils as BU
    orig = BU.run_command

    def run_patched(argv, **kw):
        argv = [a.replace("--enable-ldw-opt=false", "--enable-ldw-opt=true")
                if isinstance(a, str) else a for a in argv]
        return orig(argv, **kw)

    BU.run_command = run_patched
    _LDW_PATCHED = True


def build_nc(T=T_FULL, D=D_MODEL, h_loc=H_LOC, dh=D_HEAD, W=512,
             use_f32r=True):
    """Build the Bass module for one core's shard. Returns (nc, meta)."""
    import concourse.bass as bass
    import concourse.mybir as mybir
    import concourse.tile as tile
    from concourse import bacc
    from contextlib import ExitStack
    from collections import deque

    f32 = mybir.dt.float32
    fm = mybir.dt.float32r if use_f32r else f32
    AF = mybir.ActivationFunctionType
    ALU = mybir.AluOpType

    KN = D // 128            # k-tiles for the qkv projections
    TT = T // 128            # 128-token tiles
    assert T % W == 0 and W == 512
    NCH = T // W             # chunks
    W128 = W // 128          # s-tiles per chunk (4)
    DHL = h_loc * dh         # local head dim total (256)
    NP = h_loc // 2          # head pairs
    KO = DHL // 128          # out-proj k-tiles (2)
    VGW = dh + 1             # vg last dim: 64 V cols + ones col
    SCALE = 1.0 / float(np.sqrt(dh))

    nc = bacc.Bacc("TRN2", target_bir_lowering=False, debug=False)

    xt_d = nc.dram_tensor("xt", (128, KN, T), fm, kind="ExternalInput")
    wq_d = nc.dram_tensor("wq", (128, KN, DHL), fm, kind="ExternalInput")
    wk_d = nc.dram_tensor("wk", (128, KN, DHL), fm, kind="ExternalInput")
    wv_d = nc.dram_tensor("wv", (128, KN, DHL), fm, kind="ExternalInput")
    wg_d = nc.dram_tensor("wg", (128, DHL), fm, kind="ExternalInput")
    wo_d = nc.dram_tensor("wo", (128, KO, D), fm, kind="ExternalInput")
    mask_d = nc.dram_tensor("mask", (128, 128), fm, kind="ExternalInput")
    ones_d = nc.dram_tensor("ones", (128, TT), fm, kind="ExternalInput")
    y_d = nc.dram_tensor("y", (T, D), f32, kind="ExternalOutput")

    with ExitStack() as ctx:
        tc = ctx.enter_context(tile.TileContext(nc))
        sb_w = ctx.enter_context(tc.tile_pool(name="wts", bufs=1))
        sb_big = ctx.enter_context(tc.tile_pool(name="big", bufs=1))
        sb_e = ctx.enter_context(tc.tile_pool(name="e", bufs=3))
        sb_sig = ctx.enter_context(tc.tile_pool(name="sig", bufs=2))
        sb_nrm = ctx.enter_context(tc.tile_pool(name="nrm", bufs=1))
        sb_y = ctx.enter_context(tc.tile_pool(name="ysb", bufs=2))
        ps_b0 = ctx.enter_context(
            tc.tile_pool(name="psb0", bufs=2, space=bass.MemorySpace.PSUM))
        ps_b64 = ctx.enter_context(
            tc.tile_pool(name="psb64", bufs=2, space=bass.MemorySpace.PSUM))
        ps_u = ctx.enter_context(
            tc.tile_pool(name="psu", bufs=2, space=bass.MemorySpace.PSUM))

        # ---- persistent SBUF tensors ----
        xt = sb_big.tile([128, KN, T], fm, tag="xt")
        wq = sb_w.tile([128, KN, DHL], fm, tag="wq")
        wk = sb_w.tile([128, KN, DHL], fm, tag="wk")
        wv = sb_w.tile([128, KN, DHL], fm, tag="wv")
        wg = sb_w.tile([128, DHL], fm, tag="wg")
        wo = sb_w.tile([128, KO, D], fm, tag="wo")
        msk = sb_w.tile([128, 128], fm, tag="msk")
        qt = [sb_big.tile([128, T], fm, tag=f"qt{p}", name=f"qt{p}")
              for p in range(NP)]
        kt = [sb_big.tile([128, T], fm, tag=f"kt{p}", name=f"kt{p}")
              for p in range(NP)]
        ot = [sb_big.tile([128, T], fm, tag=f"ot{p}", name=f"ot{p}")
              for p in range(NP)]
        vg = sb_big.tile([128, TT, h_loc, VGW], fm, tag="vg")

        # ---- input DMAs: interleave per-k weight planes with xt k-planes
        # so the first matmuls can start within a few microseconds ----
        for k in range(KN):
            nc.sync.dma_start(wq[:, k, :], wq_d[:, k, :])
            nc.sync.dma_start(wk[:, k, :], wk_d[:, k, :])
            nc.sync.dma_start(xt[:, k, :], xt_d[:, k, :])
        nc.sync.dma_start(wv[:], wv_d[:])
        nc.sync.dma_start(wg[:], wg_d[:])
        nc.sync.dma_start(msk[:], mask_d[:])
        for s in range(h_loc):
            nc.sync.dma_start(vg[:, :, s, dh], ones_d[:])
        nc.sync.dma_start(wo[:], wo_d[:])

        # ---- phase-A jobs ----
        def qk_job(w_sb, dst, p, c, nch=1):
            # nch chunks share each k's LDWEIGHTS (consecutive same-lhsT
            # matmuls are elided by --enable-ldw-opt=true)
            pss = [ps_b0.tile([128, W], f32, tag="b0", name=f"qkps{cc}")
                   for cc in range(nch)]
            for k in range(KN):
                for cc in range(nch):
                    nc.tensor.matmul(
                        pss[cc][:], w_sb[:, k, 128 * p:128 * p + 128],
                        xt[:, k, (c + cc) * W:(c + cc + 1) * W],
                        start=(k == 0), stop=(k == KN - 1),
                        skip_group_check=True)
            for cc in range(nch):
                nc.vector.tensor_copy(
                    dst[:, (c + cc) * W:(c + cc + 1) * W], pss[cc][:])

        def vg_job(ti):
            # one base-0 psum tile: V in cols [0:DHL), j=0 gates in
            # [DHL:DHL+128). j=1 gates go to the base-64 pool.
            vps = ps_b0.tile([128, W], f32, tag="b0")
            for k in range(KN):
                nc.tensor.matmul(
                    vps[:, :DHL],
                    xt[:, k, 128 * ti:128 * ti + 128],
                    wv[:, k, :],
                    start=(k == 0), stop=(k == KN - 1),
                    skip_group_check=True)
            for p in range(NP):
                h = 2 * p
                nc.tensor.matmul(
                    vps[:, DHL + 64 * p:DHL + 64 * p + 64],
                    qt[p][0:64, 128 * ti:128 * ti + 128],
                    wg[0:64, dh * h:dh * h + dh],
                    start=True, stop=True, skip_group_check=True)
            gps1 = ps_b64.tile([128, W], f32, tag="b64")
            for p in range(NP):
                h = 2 * p + 1
                nc.tensor.matmul(
                    gps1[:, 64 * p:64 * p + 64],
                    qt[p][64:128, 128 * ti:128 * ti + 128],
                    wg[64:128, dh * h:dh * h + dh],
                    start=True, stop=True, skip_group_check=True)
            # sigmoid(x) = 0.5*tanh(x/2) + 0.5 (stays in the exp table set)
            sig = sb_sig.tile([128, DHL], f32, tag="sig")
            sig4 = sig[:].rearrange("p (a b c) -> p a b c", a=NP, b=2)
            nc.scalar.activation(
                sig4[:, :, 0, :],
                vps[:, DHL:DHL + 128].rearrange("p (a c) -> p a c", a=NP),
                AF.Tanh, scale=0.5)
            nc.scalar.activation(
                sig4[:, :, 1, :],
                gps1[:, 0:128].rearrange("p (a c) -> p a c", a=NP),
                AF.Tanh, scale=0.5)
            nc.vector.tensor_scalar(sig[:], sig[:], 0.5, 0.5,
                                    ALU.mult, ALU.add)
            nc.vector.tensor_mul(
                vg[:, ti, :, 0:dh],
                vps[:, :DHL].rearrange("p (h d) -> p h d", h=h_loc),
                sig[:].rearrange("p (h d) -> p h d", h=h_loc))

        # ---- phase-B inner iteration ----
        def b_iter(c, p, i, UA, UB, S):
            base = c * W128
            off = 128 * (i - base) if i >= base else 0
            sA = ps_b0.tile([128, W], f32, tag="b0")
            sB = ps_b64.tile([128, W], f32, tag="b64")
            for j, sps in ((0, sA), (1, sB)):
                nc.tensor.matmul(
                    sps[:, off:W],
                    kt[p][64 * j:64 * j + 64, 128 * i:128 * i + 128],
                    qt[p][64 * j:64 * j + 64, c * W + off:(c + 1) * W],
                    start=True, stop=True)
            es = []
            for sps in (sA, sB):
                e = sb_e.tile([128, W], fm, tag="e")
                nc.scalar.activation(e[:, off:W], sps[:, off:W], AF.Exp,
                                     scale=SCALE)
                if i >= base:
                    nc.vector.tensor_mul(e[:, off:off + 128],
                                         e[:, off:off + 128], msk[:])
                es.append(e)
            last_i = min(S - 1, base + W128 - 1)
            for j, (e, U) in ((0, (es[0], UA)), (1, (es[1], UB))):
                nc.tensor.matmul(
                    U[0:65, off:W],
                    vg[:, i, 2 * p + j, 0:65],
                    e[:, off:W],
                    start=(i == 0), stop=(i == last_i),
                    skip_group_check=True)

        def normalize(c, p, UA, UB):
            # Denominator rows live at partition 64; custom-DVE ops and
            # partition_broadcast need base-0 APs, so bounce them through
            # a cross-partition SBUF DMA.
            dtA = sb_nrm.tile([65, W], f32, tag="dtA")
            dtB = sb_nrm.tile([65, W], f32, tag="dtB")
            nc.vector.tensor_copy(dtA[64:65, :], UA[64:65, :])
            nc.vector.tensor_copy(dtB[64:65, :], UB[64:65, :])
            den = sb_nrm.tile([2, W], f32, tag="den")
            nc.sync.dma_start(den[0:1, :], dtA[64:65, :])
            nc.sync.dma_start(den[1:2, :], dtB[64:65, :])
            rec = sb_nrm.tile([2, W], f32, tag="rec")
            nc.vector.reciprocal_approx_fast(rec[:], den[:])
            recB = sb_nrm.tile([1, W], f32, tag="recB")
            nc.sync.dma_start(recB[:], rec[1:2, :])
            bcA = sb_nrm.tile([64, W], f32, tag="bcA")
            bcB = sb_nrm.tile([64, W], f32, tag="bcB")
            nc.gpsimd.partition_broadcast(bcA[:], rec[0:1, :])
            nc.gpsimd.partition_broadcast(bcB[:], recB[:])
            nc.vector.tensor_mul(ot[p][0:64, c * W:(c + 1) * W],
                                 UA[0:64, :], bcA[:])
            obB = sb_nrm.tile([64, W], fm, tag="obB")
            nc.vector.tensor_mul(obB[:], UB[0:64, :], bcB[:])
            nc.sync.dma_start(ot[p][64:128, c * W:(c + 1) * W], obB[:])

        # ---- phase-C job (one 128-token tile x one 512-col slab) ----
        def c_job(tt, n):
            yp = ps_b0.tile([128, W], f32, tag="b0")
            for kt_i in range(KO):
                nc.tensor.matmul(
                    yp[:],
                    ot[kt_i][:, 128 * tt:128 * tt + 128],
                    wo[:, kt_i, n * 512:(n + 1) * 512],
                    start=(kt_i == 0), stop=(kt_i == KO - 1),
                    skip_group_check=True)
            ysb = sb_y.tile([128, W], f32, tag="ysb")
            nc.vector.tensor_copy(ysb[:], yp[:])
            nc.sync.dma_start(
                y_d[128 * tt:128 * tt + 128, n * 512:(n + 1) * 512], ysb[:])

        def b_scores(c, p, i, S):
            base = c * W128
            off = 128 * (i - base) if i >= base else 0
            sA = ps_b0.tile([128, W], f32, tag="b0")
            sB = ps_b64.tile([128, W], f32, tag="b64")
            for j, sps in ((0, sA), (1, sB)):
                nc.tensor.matmul(
                    sps[:, off:W],
                    kt[p][64 * j:64 * j + 64, 128 * i:128 * i + 128],
                    qt[p][64 * j:64 * j + 64, c * W + off:(c + 1) * W],
                    start=True, stop=True)
            es = []
            for sps in (sA, sB):
                e = sb_e.tile([128, W], fm, tag="e")
                nc.scalar.activation(e[:, off:W], sps[:, off:W], AF.Exp,
                                     scale=SCALE)
                if i >= base:
                    nc.vector.tensor_mul(e[:, off:off + 128],
                                         e[:, off:off + 128], msk[:])
                es.append(e)
            return es, off

        def b_pv(c, p, i, UA, UB, S, es, off):
            base = c * W128
            last_i = min(S - 1, base + W128 - 1)
            for j, (e, U) in ((0, (es[0], UA)), (1, (es[1], UB))):
                nc.tensor.matmul(
                    U[0:65, off:W],
                    vg[:, i, 2 * p + j, 0:65],
                    e[:, off:W],
                    start=(i == 0), stop=(i == last_i),
                    skip_group_check=True)

        # ---- emission schedule ----
        fillers = deque()

        for p in range(NP):
            qk_job(wq, qt[p], p, 0)
        for p in range(NP):
            qk_job(wk, kt[p], p, 0)
        for ti in range(W128):
            vg_job(ti)

        for c in range(1, NCH):
            for p in range(NP):
                fillers.append(lambda p=p, c=c: qk_job(wq, qt[p], p, c))
            for p in range(NP):
                fillers.append(lambda p=p, c=c: qk_job(wk, kt[p], p, c))
            for ti in range(c * W128, (c + 1) * W128):
                fillers.append(lambda ti=ti: vg_job(ti))

        for c in range(NCH):
            S = (c + 1) * W128
            for p in range(NP):
                UA = ps_u.tile([65, W], f32, tag="UA", name="UA")
                UB = ps_u.tile([65, W], f32, tag="UB", name="UB")
                for i in range(S):
                    es, off = b_scores(c, p, i, S)
                    b_pv(c, p, i, UA, UB, S, es, off)
                    if fillers:
                        fillers.popleft()()
                normalize(c, p, UA, UB)
            for tt in range(c * W128, (c + 1) * W128):
                for n in range(D // 512):
                    fillers.append(lambda tt=tt, n=n: c_job(tt, n))
        while fillers:
            fillers.popleft()()

    nc.compile()
    meta = dict(T=T, D=D, h_loc=h_loc, dh=dh, W=W)
    return nc, meta


def prepare_core_inputs(x, W_qkv, b_qkv, W_g, W_out, b_out,
                        T=T_FULL, D=D_MODEL, h_loc=H_LOC, dh=D_HEAD):
    """Host-side sharding: returns list of per-core input dicts."""
    x = np.asarray(x, dtype=np.float32)
    W_qkv = np.asarray(W_qkv, dtype=np.float32)
    W_g = np.asarray(W_g, dtype=np.float32)
    W_out = np.asarray(W_out, dtype=np.float32)
    KN = D // 128
    DHL = h_loc * dh
    KO = DHL // 128
    n_groups = N_CORES // B
    mask = np.ascontiguousarray(
        (np.arange(128)[:, None] <= np.arange(128)[None, :]).astype(np.float32))

    in_maps = []
    for core in range(N_CORES):
        b, g = divmod(core, n_groups)
        cols = slice(DHL * g, DHL * (g + 1))
        xt = np.ascontiguousarray(
            x[b].T.reshape(KN, 128, T).transpose(1, 0, 2))
        wq = np.ascontiguousarray(
            W_qkv[:, 0 * D:1 * D][:, cols].reshape(KN, 128, DHL).transpose(1, 0, 2))
        wk = np.ascontiguousarray(
            W_qkv[:, 1 * D:2 * D][:, cols].reshape(KN, 128, DHL).transpose(1, 0, 2))
        wv = np.ascontiguousarray(
            W_qkv[:, 2 * D:3 * D][:, cols].reshape(KN, 128, DHL).transpose(1, 0, 2))
        wgh = np.zeros((128, DHL), dtype=np.float32)
        for lh in range(h_loc):
            j = lh % 2
            wgh[64 * j:64 * j + 64, dh * lh:dh * lh + dh] = W_g[h_loc * g + lh]
        wo = np.ascontiguousarray(
            W_out[DHL * g:DHL * (g + 1), :].reshape(KO, 128, D).transpose(1, 0, 2))
        in_maps.append({
            "xt": xt, "wq": wq, "wk": wk, "wv": wv,
            "wg": wgh, "wo": wo, "mask": mask,
            "ones": np.ones((128, T // 128), dtype=np.float32),
        })
    return in_maps


def gather_output(results, b_out):
    """Sum the per-core partial projections into the full output."""
    n_groups = N_CORES // B
    b_out = np.asarray(b_out, dtype=np.float32)
    outs = []
    for b in range(B):
        acc = None
        for g in range(n_groups):
            part = results[b * n_groups + g]["y"]
            acc = part.copy() if acc is None else acc + part
        outs.append(acc + b_out[None, :])
    return np.stack(outs, axis=0)


_BUILD_CACHE = {}


def _get_nc():
    key = (T_FULL, D_MODEL, H_LOC, D_HEAD)
    if key not in _BUILD_CACHE:
        _BUILD_CACHE[key] = build_nc()
    return _BUILD_CACHE[key]


def kernel(x, W_qkv, b_qkv, W_g, W_out, b_out):
    _patch_ldw_opt()
    from concourse.bass_utils import run_bass_kernel_spmd

    b_qkv = np.asarray(b_qkv, dtype=np.float32)
    assert not np.any(b_qkv), "nonzero b_qkv not supported by this build"
    nc, _ = _get_nc()
    in_maps = prepare_core_inputs(x, W_qkv, b_qkv, W_g, W_out, b_out)
    res = run_bass_kernel_spmd(nc, in_maps, core_ids=list(range(N_CORES)))
    return gather_output(res.results, b_out).astype(np.float32)
